# revision 11
# baseline (speedup 1.0000x reference)
"""Trainium2 Bass kernel for nn_MultiScaleDHSM (multi-scale diagonal-SSM LM block).

Strategy (zero-communication SPMD over 8 cores):
  core c owns tokens [512*q, 512*(q+1)) of batch b, where b=c//4, q=c%4.
  Device keeps everything feature-major [feature, token] so every per-feature
  vector (biases, D, LN folds) is a per-partition scalar.  The sequential
  recurrence s_t = A*s_{t-1} + u_t runs on the HW tensor_tensor_scan op over
  the core's right-aligned token prefix (front zero-padded so the 512-token
  "own window" sits at a static offset in an identical program on all cores).
  LayerNorm stats (reduction over features = partitions) are computed with a
  ones-matmul on the PE, which both reduces and broadcasts across partitions.
  LN scales fold into the following matmul weights on the host (g -> Wo,
  gf -> Wh); per-feature biases are applied as per-partition ACT biases.
"""

import os
from contextlib import ExitStack

import ml_dtypes
import numpy as np

import concourse.bass as bass
import concourse.mybir as mybir
import concourse.tile as tile
from concourse import bacc
from concourse.bass import ds, ts
from concourse.bass_utils import run_bass_kernel_spmd

B, S, H, V = 2, 2048, 1024, 32000
SDS = [64, 128, 256]
NP = 512  # packed state dim: [L0:0-64 | pad:64-128 | L1:128-256 | L2:256-512]
SLOT = [0, 128, 256]
NT_OF_LAYER = [[0], [1], [2, 3]]  # which 128-row n-tiles belong to each layer
TPC = 512  # tokens per core
EPS = 1e-5
VG = 1000  # head vocab group (streamed Wh slice width)
VC = 500   # head vocab chunk (one PSUM bank, <=512 fp32)
NGRP = V // VG
NCHK = VG // VC

F32 = mybir.dt.float32
BF16 = mybir.dt.bfloat16
BF = ml_dtypes.bfloat16
AF = mybir.ActivationFunctionType
OP = mybir.AluOpType

last_exec_time_ns = None
last_bass_results = None

_prog_cache = {}


def _layernorm_stats(nc, psum, sb, ones, epst, ysrc, sqsrc):
    """ysrc/sqsrc: [128, 8, TPC] bf16 tiles. Returns (rstd, mur) [128, TPC] f32."""
    pm1 = psum.tile([128, TPC], F32, tag="mm", bufs=4, name="pm1")
    pm2 = psum.tile([128, TPC], F32, tag="mm", bufs=4, name="pm2")
    for ho in range(8):
        nc.tensor.matmul(pm1[:], ones[:], ysrc[:, ho, :], start=(ho == 0), stop=(ho == 7))
    for ho in range(8):
        nc.tensor.matmul(pm2[:], ones[:], sqsrc[:, ho, :], start=(ho == 0), stop=(ho == 7))
    musq = sb.tile([128, TPC], F32, tag="musq")
    nc.scalar.activation(musq[:], pm1[:], AF.Square, scale=1.0 / H)
    var = sb.tile([128, TPC], F32, tag="var")
    nc.vector.scalar_tensor_tensor(var[:], pm2[:], 1.0 / H, musq[:], OP.mult, OP.subtract)
    sd = sb.tile([128, TPC], F32, tag="sd")
    nc.scalar.activation(sd[:], var[:], AF.Sqrt, bias=epst[:, 0:1])
    rstd = sb.tile([128, TPC], F32, tag="rstd")
    nc.vector.reciprocal(rstd[:], sd[:])
    mur = sb.tile([128, TPC], F32, tag="mur")
    nc.vector.scalar_tensor_tensor(mur[:], pm1[:], 1.0 / H, rstd[:], OP.mult, OP.mult)
    return rstd, mur


def _body(tc, io):
    nc = tc.nc
    with ExitStack() as ctx:
        sb = ctx.enter_context(tc.tile_pool(name="sb", bufs=1))
        sb2 = ctx.enter_context(tc.tile_pool(name="sb2", bufs=2))
        sb3 = ctx.enter_context(tc.tile_pool(name="sb3", bufs=3))
        psum = ctx.enter_context(tc.tile_pool(name="ps", bufs=4, space="PSUM"))

        r8 = lambda ap: ap.rearrange("(r p) t -> p r t", p=128)

        def dma_in(pool, name, src_ap, shape, dtype):
            t = pool.tile(shape, dtype, tag=name, name=name)
            nc.sync.dma_start(t[:], src_ap)
            return t

        # ---- persistent small tensors ----
        embm = dma_in(sb, "embm", r8(io["embm"]), [128, 8, TPC], BF16)
        wgb = dma_in(sb2, "w16", r8(io["wgb"]), [128, 8, 2 * NP], BF16)
        bgp = dma_in(sb, "bgp", io["bgp"].rearrange("(n p) o -> p (n o)", p=128), [128, 4], F32)
        apk = dma_in(sb, "apk", io["apk"].rearrange("(n p) o -> p (n o)", p=128), [128, 4], F32)
        wct = dma_in(sb, "wct", io["wct"].rearrange("(n p) h -> p n h", p=128), [128, 4, H], BF16)
        d1 = dma_in(sb, "d1", r8(io["d1"]), [128, 8, 3], F32)
        bop = dma_in(sb, "bop", r8(io["bop"]), [128, 8, 3], F32)
        bfv = dma_in(sb, "bfv", r8(io["bfv"]), [128, 8, 1], F32)
        ones = sb.tile([128, 128], BF16, tag="ones")
        nc.gpsimd.memset(ones[:], 1.0)
        epst = sb.tile([128, 1], F32, tag="epst")
        nc.gpsimd.memset(epst[:], EPS)

        # A broadcast along a 512-token chunk, per n-tile (scan data0)
        abc = sb.tile([128, 4, 512], F32, tag="abc")
        nc.gpsimd.memset(abc[:], 0.0)
        for nt in range(4):
            nc.scalar.activation(abc[:, nt, :], abc[:, nt, :], AF.Identity, bias=apk[:, nt:nt + 1])

        # ---- stage 1+2: u = sigmoid(emb@WgT + bg) * (emb@WbT); chained HW scan ----
        states = None
        prev_states = None
        for t4 in range(4):
            et = sb2.tile([128, 8, 512], BF16, tag="e8", name=f"et{t4}")
            nc.sync.dma_start(et[:], r8(io["embt"])[:, :, ts(t4, 512)])
            st = sb2.tile([128, 4, 512], BF16, tag="stc", name=f"st{t4}")
            for nt in range(4):
                pg = psum.tile([128, 512], F32, tag="mm", bufs=4, name=f"pg{t4}_{nt}")
                pb = psum.tile([128, 512], F32, tag="mm", bufs=4, name=f"pb{t4}_{nt}")
                for r in range(8):
                    nc.tensor.matmul(pg[:], wgb[:, r, ts(nt, 128)], et[:, r, :],
                                     start=(r == 0), stop=(r == 7))
                for r in range(8):
                    nc.tensor.matmul(pb[:], wgb[:, r, ts(4 + nt, 128)], et[:, r, :],
                                     start=(r == 0), stop=(r == 7))
                gate = sb2.tile([128, 512], BF16, tag="gate")
                nc.scalar.activation(gate[:], pg[:], AF.Sigmoid, bias=bgp[:, nt:nt + 1])
                uc = sb3.tile([128, 512], BF16, tag="uc")
                nc.vector.tensor_mul(uc[:], gate[:], pb[:])
                init = 0.0 if t4 == 0 else prev_states[:, nt, 511:512]
                nc.vector.tensor_tensor_scan(st[:, nt, :], abc[:, nt, :], uc[:],
                                             init, OP.mult, OP.add)
            prev_states = st
        states = prev_states  # [128, 4, 512] bf16: my-window states

        # ---- stage 3: per layer: y = states@WcT + (D+1)*x ; LN1 ; o = normed@Wo' ----
        combined = sb.tile([128, 24, TPC], BF16, tag="combined")
        for i in range(3):
            yT = sb2.tile([128, 8, TPC], BF16, tag="ysq", name=f"yT{i}")
            sq = sb2.tile([128, 8, TPC], BF16, tag="ysq", name=f"sq{i}")
            tls = NT_OF_LAYER[i]
            for ho in range(8):
                py = psum.tile([128, TPC], F32, tag="mm", bufs=4, name=f"py{i}_{ho}")
                for j, nt in enumerate(tls):
                    nc.tensor.matmul(py[:], wct[:, nt, ts(ho, 128)], states[:, nt, :],
                                     start=(j == 0), stop=(j == len(tls) - 1))
                nc.vector.scalar_tensor_tensor(yT[:, ho, :], embm[:, ho, :], d1[:, ho, i:i + 1],
                                               py[:], OP.mult, OP.add)
                nc.scalar.activation(sq[:, ho, :], yT[:, ho, :], AF.Square)
            rstd, mur = _layernorm_stats(nc, psum, sb, ones, epst, yT, sq)
            normed = sb2.tile([128, 8, TPC], BF16, tag="nrm", name=f"nrm{i}")
            for ho in range(8):
                tmp = sb2.tile([128, TPC], F32, tag="tmp")
                nc.vector.tensor_mul(tmp[:], yT[:, ho, :], rstd[:])
                nc.vector.tensor_sub(normed[:, ho, :], tmp[:], mur[:])
            for ho in range(8):
                woth = sb2.tile([128, 8, 128], BF16, tag="woth")
                nc.sync.dma_start(woth[:], r8(io["wot"])[:, :, ds(i * H + ho * 128, 128)])
                po = psum.tile([128, TPC], F32, tag="mm", bufs=4, name=f"po{i}_{ho}")
                for hi in range(8):
                    nc.tensor.matmul(po[:], woth[:, hi, :], normed[:, hi, :],
                                     start=(hi == 0), stop=(hi == 7))
                nc.scalar.activation(combined[:, i * 8 + ho, :], po[:], AF.Identity,
                                     bias=bop[:, ho, i:i + 1])

        # ---- stage 4: fused = LN2(combined@WfT + bf + emb) ----
        fused = sb2.tile([128, 8, TPC], BF16, tag="ysq", name="fused")
        sq2 = sb2.tile([128, 8, TPC], BF16, tag="ysq", name="sq2")
        for ho in range(8):
            wfh = sb2.tile([128, 24, 128], BF16, tag="wfh")
            nc.sync.dma_start(wfh[:], io["wft"].rearrange("(k p) m -> p k m", p=128)[:, :, ts(ho, 128)])
            pf = psum.tile([128, TPC], F32, tag="mm", bufs=4, name=f"pf{ho}")
            for k in range(24):
                nc.tensor.matmul(pf[:], wfh[:, k, :], combined[:, k, :],
                                 start=(k == 0), stop=(k == 23))
            nc.vector.scalar_tensor_tensor(fused[:, ho, :], pf[:], bfv[:, ho, 0:1],
                                           embm[:, ho, :], OP.add, OP.add)
            nc.scalar.activation(sq2[:, ho, :], fused[:, ho, :], AF.Square)
        rstd2, mur2 = _layernorm_stats(nc, psum, sb, ones, epst, fused, sq2)
        norm2 = sb.tile([128, 8, TPC], BF16, tag="norm2")
        for ho in range(8):
            tmp = sb2.tile([128, TPC], F32, tag="tmp")
            nc.vector.tensor_mul(tmp[:], fused[:, ho, :], rstd2[:])
            nc.vector.tensor_sub(norm2[:, ho, :], tmp[:], mur2[:])

        # ---- stage 5: head: logits[t, v] = norm2.T @ Wh' (+ bh') ----
        bh_nonzero = io.get("bhp") is not None
        if bh_nonzero:
            e0row = sb.tile([128, 128], BF16, tag="e0row")
            nc.gpsimd.memset(e0row[:], 0.0)
            nc.gpsimd.memset(e0row[0:1, :], 1.0)
        for grp in range(NGRP):
            whg = sb2.tile([128, 8, VG], BF16, tag="w16", name=f"whg{grp}")
            nc.sync.dma_start(whg[:], r8(io["wht"])[:, :, ts(grp, VG)])
            if bh_nonzero:
                bhg = sb2.tile([128, VG], BF16, tag="bhg")
                nc.gpsimd.memset(bhg[:], 0.0)
                nc.sync.dma_start(bhg[0:1, :], io["bhp"][:, ts(grp, VG)])
            for t in range(4):
                osb = sb2.tile([128, VG], F32, tag="osb")
                pcs = [psum.tile([128, VC], F32, tag="hc", bufs=4, name=f"hc{grp}_{t}_{c}")
                       for c in range(NCHK)]
                for k in range(8):
                    for c in range(NCHK):
                        nc.tensor.matmul(pcs[c][:], norm2[:, k, ts(t, 128)], whg[:, k, ts(c, VC)],
                                         start=(k == 0), stop=(k == 7 and not bh_nonzero))
                if bh_nonzero:
                    for c in range(NCHK):
                        nc.tensor.matmul(pcs[c][:], e0row[:], bhg[:, ts(c, VC)],
                                         start=False, stop=True)
                for c in range(NCHK):
                    if c % 2 == 0:
                        nc.scalar.copy(osb[:, ts(c, VC)], pcs[c][:])
                    else:
                        nc.vector.tensor_copy(osb[:, ts(c, VC)], pcs[c][:])
                nc.sync.dma_start(io["out"][ts(t, 128), ts(grp, VG)], osb[:])


def _get_program(bh_nonzero):
    key = bool(bh_nonzero)
    if key in _prog_cache:
        return _prog_cache[key]
    nc = bacc.Bacc("TRN2", target_bir_lowering=False, debug=False, num_devices=8)
    io = {}

    def din(name, shape, dtype):
        io[name] = nc.dram_tensor(name, list(shape), dtype, kind="ExternalInput").ap()

    din("embt", (H, S), BF16)
    din("embm", (H, TPC), BF16)
    din("wgb", (H, 2 * NP), BF16)
    din("bgp", (NP, 1), F32)
    din("apk", (NP, 1), F32)
    din("wct", (NP, H), BF16)
    din("d1", (H, 3), F32)
    din("wot", (H, 3 * H), BF16)
    din("bop", (H, 3), F32)
    din("wft", (3 * H, H), BF16)
    din("bfv", (H, 1), F32)
    din("wht", (H, V), BF16)
    if bh_nonzero:
        din("bhp", (1, V), BF16)
    else:
        io["bhp"] = None
    io["out"] = nc.dram_tensor("out", [TPC, V], F32, kind="ExternalOutput").ap()

    with tile.TileContext(nc) as tc:
        _body(tc, io)
    nc.compile()
    _prog_cache[key] = nc
    return nc


def _prepare(inputs):
    f32 = np.float32
    E = np.asarray(inputs["E"], f32)
    x = np.asarray(inputs["x"]).astype(np.int64)
    emb = E[x]  # [B, S, H]

    Wg = [np.asarray(inputs[f"Wg{i}"], f32) for i in range(3)]
    Wb = [np.asarray(inputs[f"Wb{i}"], f32) for i in range(3)]
    Wc = [np.asarray(inputs[f"Wc{i}"], f32) for i in range(3)]
    Dv = [np.asarray(inputs[f"D{i}"], f32) for i in range(3)]
    bg = [np.asarray(inputs[f"bg{i}"], f32) for i in range(3)]
    Wo = [np.asarray(inputs[f"Wo{i}"], f32) for i in range(3)]
    bo = [np.asarray(inputs[f"bo{i}"], f32) for i in range(3)]
    gv = [np.asarray(inputs[f"g{i}"], f32) for i in range(3)]
    be = [np.asarray(inputs[f"be{i}"], f32) for i in range(3)]
    Alog = [np.asarray(inputs[f"A{i}"], f32) for i in range(3)]
    Wf = np.asarray(inputs["Wf"], f32)
    bf = np.asarray(inputs["bf"], f32)
    gf = np.asarray(inputs["gf"], f32)
    bef = np.asarray(inputs["bef"], f32)
    Wh = np.asarray(inputs["Wh"], f32)
    bh = np.asarray(inputs["bh"], f32)

    wgb = np.zeros((H, 2 * NP), f32)
    bgp = np.zeros((NP, 1), f32)
    apk = np.zeros((NP, 1), f32)
    wct = np.zeros((NP, H), f32)
    for i, N in enumerate(SDS):
        s = SLOT[i]
        wgb[:, s:s + N] = Wg[i].T
        wgb[:, NP + s:NP + s + N] = Wb[i].T
        bgp[s:s + N, 0] = bg[i]
        apk[s:s + N, 0] = np.exp(Alog[i])
        wct[s:s + N, :] = Wc[i].T
    d1 = np.stack([Dv[i] + 1.0 for i in range(3)], axis=1)  # [H, 3]
    wot = np.concatenate([(Wo[i] * gv[i][None, :]).T for i in range(3)], axis=1)  # [H, 3H]
    bop = np.stack([Wo[i] @ be[i] + bo[i] for i in range(3)], axis=1)  # [H, 3]
    wft = Wf.T.copy()  # [3H, H]
    wht = (Wh * gf[None, :]).T.copy()  # [H, V]
    bhp = Wh @ bef + bh  # [V]
    bh_nonzero = bool(np.abs(bhp).max() > 0)

    shared = {
        "wgb": wgb.astype(BF),
        "bgp": bgp,
        "apk": apk,
        "wct": wct.astype(BF),
        "d1": d1,
        "wot": wot.astype(BF),
        "bop": bop,
        "wft": wft.astype(BF),
        "bfv": bf.reshape(H, 1),
        "wht": wht.astype(BF),
    }
    if bh_nonzero:
        shared["bhp"] = bhp.reshape(1, V).astype(BF)

    in_maps = []
    for c in range(8):
        b, q = c // 4, c % 4
        plen = TPC * (q + 1)
        et = np.zeros((H, S), BF)
        et[:, S - plen:] = emb[b, :plen].T.astype(BF)
        em = np.ascontiguousarray(emb[b, q * TPC:(q + 1) * TPC].T).astype(BF)
        m = dict(shared)
        m["embt"] = et
        m["embm"] = em
        in_maps.append(m)
    return in_maps, bh_nonzero


def kernel(**inputs):
    global last_exec_time_ns, last_bass_results
    in_maps, bh_nonzero = _prepare(inputs)
    nc = _get_program(bh_nonzero)
    trace = os.environ.get("BASS_KERNEL_TRACE") == "1"
    tmpdir = os.environ.get("BASS_TRACE_DIR") or None
    res = run_bass_kernel_spmd(nc, in_maps, core_ids=list(range(8)), trace=trace,
                               tmpdir=tmpdir)
    last_exec_time_ns = res.exec_time_ns
    last_bass_results = res

    out = np.empty((B, S, V), np.float32)
    for c in range(8):
        b, q = c // 4, c % 4
        out[b, q * TPC:(q + 1) * TPC, :] = res.results[c]["out"]
    return out


# revision 17
# speedup vs baseline: 1.1021x; 1.1021x over previous
"""Trainium2 Bass kernel for nn_MultiScaleDHSM (multi-scale diagonal-SSM LM block).

Strategy (zero-communication SPMD over 8 cores):
  core c owns tokens [512*q, 512*(q+1)) of batch b, where b=c//4, q=c%4.
  Device keeps everything feature-major [feature, token] so every per-feature
  vector (biases, D, LN folds) is a per-partition scalar.  The sequential
  recurrence s_t = A*s_{t-1} + u_t runs on the HW tensor_tensor_scan op over
  the core's right-aligned token prefix (front zero-padded so the 512-token
  "own window" sits at a static offset in an identical program on all cores).
  LayerNorm stats (reduction over features = partitions) are computed with a
  ones-matmul on the PE, which both reduces and broadcasts across partitions.
  LN scales fold into the following matmul weights on the host (g -> Wo,
  gf -> Wh); per-feature biases are applied as per-partition ACT biases.
"""

import os
from contextlib import ExitStack

import ml_dtypes
import numpy as np

import concourse.bass as bass
import concourse.mybir as mybir
import concourse.tile as tile
from concourse import bacc
from concourse.bass import ds, ts
from concourse.bass_utils import run_bass_kernel_spmd

B, S, H, V = 2, 2048, 1024, 32000
SDS = [64, 128, 256]
NP = 512  # packed state dim: [L0:0-64 | pad:64-128 | L1:128-256 | L2:256-512]
SLOT = [0, 128, 256]
NT_OF_LAYER = [[0], [1], [2, 3]]  # which 128-row n-tiles belong to each layer
TPC = 512  # tokens per core
EPS = 1e-5
VG = 1000  # head vocab group (streamed Wh slice width)
VC = 500   # head vocab chunk (one PSUM bank, <=512 fp32)
NGRP = V // VG
NCHK = VG // VC

F32 = mybir.dt.float32
BF16 = mybir.dt.bfloat16
BF = ml_dtypes.bfloat16
AF = mybir.ActivationFunctionType
OP = mybir.AluOpType

last_exec_time_ns = None
last_bass_results = None

_prog_cache = {}


def _layernorm_stats(nc, psum, sb, ones, epst, ysrc, sqsrc):
    """ysrc/sqsrc: [128, 8, TPC] bf16 tiles. Returns (rstd, mur) [128, TPC] f32."""
    pm1 = psum.tile([128, TPC], F32, tag="ps", bufs=8, name="pm1")
    pm2 = psum.tile([128, TPC], F32, tag="ps", bufs=8, name="pm2")
    for ho in range(8):
        nc.tensor.matmul(pm1[:], ones[:], ysrc[:, ho, :], start=(ho == 0), stop=(ho == 7))
    for ho in range(8):
        nc.tensor.matmul(pm2[:], ones[:], sqsrc[:, ho, :], start=(ho == 0), stop=(ho == 7))
    musq = sb.tile([128, TPC], F32, tag="musq")
    nc.scalar.activation(musq[:], pm1[:], AF.Square, scale=1.0 / H)
    var = sb.tile([128, TPC], F32, tag="var")
    nc.vector.scalar_tensor_tensor(var[:], pm2[:], 1.0 / H, musq[:], OP.mult, OP.subtract)
    sd = sb.tile([128, TPC], F32, tag="sd")
    nc.scalar.activation(sd[:], var[:], AF.Sqrt, bias=epst[:, 0:1])
    rstd = sb.tile([128, TPC], F32, tag="rstd")
    nc.vector.reciprocal(rstd[:], sd[:])
    mur = sb.tile([128, TPC], F32, tag="mur")
    nc.vector.scalar_tensor_tensor(mur[:], pm1[:], 1.0 / H, rstd[:], OP.mult, OP.mult)
    return rstd, mur


def _body(tc, io):
    nc = tc.nc
    with ExitStack() as ctx:
        sb = ctx.enter_context(tc.tile_pool(name="sb", bufs=1))
        sb2 = ctx.enter_context(tc.tile_pool(name="sb2", bufs=2))
        sb3 = ctx.enter_context(tc.tile_pool(name="sb3", bufs=3))
        psum = ctx.enter_context(tc.tile_pool(name="ps", bufs=4, space="PSUM"))

        r8 = lambda ap: ap.rearrange("(r p) t -> p r t", p=128)

        def dma_in(pool, name, src_ap, shape, dtype, bufs=None):
            kw = {"bufs": bufs} if bufs else {}
            t = pool.tile(shape, dtype, tag=name, name=name, **kw)
            nc.sync.dma_start(t[:], src_ap)
            return t

        # ---- persistent small tensors (stage-1 needs first; rest after) ----
        wgb = dma_in(sb2, "w16", r8(io["wgb"]), [128, 8, 2 * NP], BF16, bufs=3)
        bgp = dma_in(sb, "bgp", io["bgp"].rearrange("(n p) o -> p (n o)", p=128), [128, 4], F32)
        apk = dma_in(sb, "apk", io["apk"].rearrange("(n p) o -> p (n o)", p=128), [128, 4], F32)
        ones = sb.tile([128, 128], BF16, tag="ones")
        nc.gpsimd.memset(ones[:], 1.0)
        epst = sb.tile([128, 1], F32, tag="epst")
        nc.gpsimd.memset(epst[:], EPS)

        # ---- stage 1+2: u = sigmoid(emb@WgT + bg) * (emb@WbT); chained HW scan ----
        states = None
        prev_states = None
        embm = None  # the t4=3 chunk doubles as the own-window embedding
        for t4 in range(4):
            if t4 < 3:
                et = sb2.tile([128, 8, 512], BF16, tag="e8", name=f"et{t4}")
            else:
                et = sb.tile([128, 8, 512], BF16, tag="embm", name="et3")
                embm = et
            nc.sync.dma_start(et[:], r8(io["embt"])[:, :, ts(t4, 512)])
            st = sb2.tile([128, 4, 512], BF16, tag="stc", name=f"st{t4}")
            for nt in range(4):
                pg = psum.tile([128, 512], F32, tag="ps", bufs=8, name=f"pg{t4}_{nt}")
                pb = psum.tile([128, 512], F32, tag="ps", bufs=8, name=f"pb{t4}_{nt}")
                for r in range(8):
                    nc.tensor.matmul(pg[:], wgb[:, r, ts(nt, 128)], et[:, r, :],
                                     start=(r == 0), stop=(r == 7))
                for r in range(8):
                    nc.tensor.matmul(pb[:], wgb[:, r, ts(4 + nt, 128)], et[:, r, :],
                                     start=(r == 0), stop=(r == 7))
                gate = sb2.tile([128, 512], BF16, tag="gate")
                nc.scalar.activation(gate[:], pg[:], AF.Sigmoid, bias=bgp[:, nt:nt + 1])
                uc = sb3.tile([128, 512], BF16, tag="uc")
                nc.vector.tensor_mul(uc[:], gate[:], pb[:])
                init = 0.0 if t4 == 0 else prev_states[:, nt, 511:512]
                nc.vector.tensor_tensor_scan(st[:, nt, :],
                                             apk[:, nt:nt + 1].to_broadcast([128, 512]),
                                             uc[:], init, OP.mult, OP.add)
            prev_states = st
        states = prev_states  # [128, 4, 512] bf16: my-window states

        # remaining persistent small tensors (first used in stage 3/4)
        wct = dma_in(sb, "wct", io["wct"].rearrange("(n p) h -> p n h", p=128), [128, 4, H], BF16)
        d1 = dma_in(sb, "d1", r8(io["d1"]), [128, 8, 3], F32)
        bop = dma_in(sb, "bop", r8(io["bop"]), [128, 8, 3], F32)
        bfv = dma_in(sb, "bfv", r8(io["bfv"]), [128, 8, 1], F32)

        # ---- stage 3: per layer: y = states@WcT + (D+1)*x ; LN1 ; o = normed@Wo' ----
        combined = sb.tile([128, 24, TPC], BF16, tag="combined")
        for i in range(3):
            yT = sb2.tile([128, 8, TPC], BF16, tag="ysq", bufs=4, name=f"yT{i}")
            sq = sb2.tile([128, 8, TPC], BF16, tag="ysq", bufs=4, name=f"sq{i}")
            tls = NT_OF_LAYER[i]
            for ho in range(8):
                py = psum.tile([128, TPC], F32, tag="ps", bufs=8, name=f"py{i}_{ho}")
                for j, nt in enumerate(tls):
                    nc.tensor.matmul(py[:], wct[:, nt, ts(ho, 128)], states[:, nt, :],
                                     start=(j == 0), stop=(j == len(tls) - 1))
                nc.vector.scalar_tensor_tensor(yT[:, ho, :], embm[:, ho, :], d1[:, ho, i:i + 1],
                                               py[:], OP.mult, OP.add)
                nc.scalar.activation(sq[:, ho, :], yT[:, ho, :], AF.Square)
            rstd, mur = _layernorm_stats(nc, psum, sb, ones, epst, yT, sq)
            normed = sb2.tile([128, 8, TPC], BF16, tag="nrm", name=f"nrm{i}")
            for ho in range(8):
                tmp = sb2.tile([128, TPC], F32, tag="tmp")
                nc.vector.tensor_mul(tmp[:], yT[:, ho, :], rstd[:])
                nc.vector.tensor_sub(normed[:, ho, :], tmp[:], mur[:])
            for ho in range(8):
                woth = sb2.tile([128, 24, 128], BF16, tag="wst", name=f"woth{i}_{ho}")[:, :8, :]
                nc.sync.dma_start(woth[:], r8(io["wot"])[:, :, ds(i * H + ho * 128, 128)])
                po = psum.tile([128, TPC], F32, tag="ps", bufs=8, name=f"po{i}_{ho}")
                for hi in range(8):
                    nc.tensor.matmul(po[:], woth[:, hi, :], normed[:, hi, :],
                                     start=(hi == 0), stop=(hi == 7))
                nc.scalar.activation(combined[:, i * 8 + ho, :], po[:], AF.Identity,
                                     bias=bop[:, ho, i:i + 1])

        # ---- stage 4: fused = LN2(combined@WfT + bf + emb) ----
        fused = sb2.tile([128, 8, TPC], BF16, tag="ysq", bufs=4, name="fused")
        sq2 = sb2.tile([128, 8, TPC], BF16, tag="ysq", bufs=4, name="sq2")
        for ho in range(8):
            wfh = sb2.tile([128, 24, 128], BF16, tag="wst", name=f"wfh{ho}")
            nc.sync.dma_start(wfh[:], io["wft"].rearrange("(k p) m -> p k m", p=128)[:, :, ts(ho, 128)])
            pf = psum.tile([128, TPC], F32, tag="ps", bufs=8, name=f"pf{ho}")
            for k in range(24):
                nc.tensor.matmul(pf[:], wfh[:, k, :], combined[:, k, :],
                                 start=(k == 0), stop=(k == 23))
            nc.vector.scalar_tensor_tensor(fused[:, ho, :], pf[:], bfv[:, ho, 0:1],
                                           embm[:, ho, :], OP.add, OP.add)
            nc.scalar.activation(sq2[:, ho, :], fused[:, ho, :], AF.Square)
        rstd2, mur2 = _layernorm_stats(nc, psum, sb, ones, epst, fused, sq2)
        norm2 = sb.tile([128, 8, TPC], BF16, tag="norm2")
        for ho in range(8):
            tmp = sb2.tile([128, TPC], F32, tag="tmp")
            nc.vector.tensor_mul(tmp[:], fused[:, ho, :], rstd2[:])
            nc.vector.tensor_sub(norm2[:, ho, :], tmp[:], mur2[:])

        # ---- stage 5: head: logits[t, v] = norm2.T @ Wh' (+ bh') ----
        bh_nonzero = io.get("bhp") is not None
        if bh_nonzero:
            e0row = sb.tile([128, 128], BF16, tag="e0row")
            nc.gpsimd.memset(e0row[:], 0.0)
            nc.gpsimd.memset(e0row[0:1, :], 1.0)
        for grp in range(NGRP):
            whg = sb2.tile([128, 8, VG], BF16, tag="w16", bufs=3, name=f"whg{grp}")
            nc.sync.dma_start(whg[:], r8(io["wht"])[:, :, ts(grp, VG)])
            if bh_nonzero:
                bhg = sb2.tile([128, VG], BF16, tag="bhg")
                nc.gpsimd.memset(bhg[:], 0.0)
                nc.sync.dma_start(bhg[0:1, :], io["bhp"][:, ts(grp, VG)])
            for t in range(4):
                osb = sb2.tile([128, VG], F32, tag="osb")
                pcs = [psum.tile([128, VC], F32, tag="ps", bufs=8, name=f"hc{grp}_{t}_{c}")
                       for c in range(NCHK)]
                for k in range(8):
                    for c in range(NCHK):
                        nc.tensor.matmul(pcs[c][:], norm2[:, k, ts(t, 128)], whg[:, k, ts(c, VC)],
                                         start=(k == 0), stop=(k == 7 and not bh_nonzero))
                if bh_nonzero:
                    for c in range(NCHK):
                        nc.tensor.matmul(pcs[c][:], e0row[:], bhg[:, ts(c, VC)],
                                         start=False, stop=True)
                for c in range(NCHK):
                    if c % 2 == 0:
                        nc.scalar.copy(osb[:, ts(c, VC)], pcs[c][:])
                    else:
                        nc.vector.tensor_copy(osb[:, ts(c, VC)], pcs[c][:])
                nc.sync.dma_start(io["out"][ts(t, 128), ts(grp, VG)], osb[:])


def _get_program(bh_nonzero):
    key = bool(bh_nonzero)
    if key in _prog_cache:
        return _prog_cache[key]
    nc = bacc.Bacc("TRN2", target_bir_lowering=False, debug=False, num_devices=8)
    io = {}

    def din(name, shape, dtype):
        io[name] = nc.dram_tensor(name, list(shape), dtype, kind="ExternalInput").ap()

    din("embt", (H, S), BF16)
    din("embm", (H, TPC), BF16)
    din("wgb", (H, 2 * NP), BF16)
    din("bgp", (NP, 1), F32)
    din("apk", (NP, 1), F32)
    din("wct", (NP, H), BF16)
    din("d1", (H, 3), F32)
    din("wot", (H, 3 * H), BF16)
    din("bop", (H, 3), F32)
    din("wft", (3 * H, H), BF16)
    din("bfv", (H, 1), F32)
    din("wht", (H, V), BF16)
    if bh_nonzero:
        din("bhp", (1, V), BF16)
    else:
        io["bhp"] = None
    io["out"] = nc.dram_tensor("out", [TPC, V], F32, kind="ExternalOutput").ap()

    with tile.TileContext(nc) as tc:
        _body(tc, io)
    nc.compile()
    _prog_cache[key] = nc
    return nc


def _prepare(inputs):
    f32 = np.float32
    E = np.asarray(inputs["E"], f32)
    x = np.asarray(inputs["x"]).astype(np.int64)
    emb = E[x]  # [B, S, H]

    Wg = [np.asarray(inputs[f"Wg{i}"], f32) for i in range(3)]
    Wb = [np.asarray(inputs[f"Wb{i}"], f32) for i in range(3)]
    Wc = [np.asarray(inputs[f"Wc{i}"], f32) for i in range(3)]
    Dv = [np.asarray(inputs[f"D{i}"], f32) for i in range(3)]
    bg = [np.asarray(inputs[f"bg{i}"], f32) for i in range(3)]
    Wo = [np.asarray(inputs[f"Wo{i}"], f32) for i in range(3)]
    bo = [np.asarray(inputs[f"bo{i}"], f32) for i in range(3)]
    gv = [np.asarray(inputs[f"g{i}"], f32) for i in range(3)]
    be = [np.asarray(inputs[f"be{i}"], f32) for i in range(3)]
    Alog = [np.asarray(inputs[f"A{i}"], f32) for i in range(3)]
    Wf = np.asarray(inputs["Wf"], f32)
    bf = np.asarray(inputs["bf"], f32)
    gf = np.asarray(inputs["gf"], f32)
    bef = np.asarray(inputs["bef"], f32)
    Wh = np.asarray(inputs["Wh"], f32)
    bh = np.asarray(inputs["bh"], f32)

    wgb = np.zeros((H, 2 * NP), f32)
    bgp = np.zeros((NP, 1), f32)
    apk = np.zeros((NP, 1), f32)
    wct = np.zeros((NP, H), f32)
    for i, N in enumerate(SDS):
        s = SLOT[i]
        wgb[:, s:s + N] = Wg[i].T
        wgb[:, NP + s:NP + s + N] = Wb[i].T
        bgp[s:s + N, 0] = bg[i]
        apk[s:s + N, 0] = np.exp(Alog[i])
        wct[s:s + N, :] = Wc[i].T
    d1 = np.stack([Dv[i] + 1.0 for i in range(3)], axis=1)  # [H, 3]
    wot = np.concatenate([(Wo[i] * gv[i][None, :]).T for i in range(3)], axis=1)  # [H, 3H]
    bop = np.stack([Wo[i] @ be[i] + bo[i] for i in range(3)], axis=1)  # [H, 3]
    wft = Wf.T.copy()  # [3H, H]
    wht = (Wh * gf[None, :]).T.copy()  # [H, V]
    bhp = Wh @ bef + bh  # [V]
    bh_nonzero = bool(np.abs(bhp).max() > 0)

    shared = {
        "wgb": wgb.astype(BF),
        "bgp": bgp,
        "apk": apk,
        "wct": wct.astype(BF),
        "d1": d1,
        "wot": wot.astype(BF),
        "bop": bop,
        "wft": wft.astype(BF),
        "bfv": bf.reshape(H, 1),
        "wht": wht.astype(BF),
    }
    if bh_nonzero:
        shared["bhp"] = bhp.reshape(1, V).astype(BF)

    in_maps = []
    for c in range(8):
        b, q = c // 4, c % 4
        plen = TPC * (q + 1)
        et = np.zeros((H, S), BF)
        et[:, S - plen:] = emb[b, :plen].T.astype(BF)
        em = np.ascontiguousarray(emb[b, q * TPC:(q + 1) * TPC].T).astype(BF)
        m = dict(shared)
        m["embt"] = et
        m["embm"] = em
        in_maps.append(m)
    return in_maps, bh_nonzero


def kernel(**inputs):
    global last_exec_time_ns, last_bass_results
    in_maps, bh_nonzero = _prepare(inputs)
    nc = _get_program(bh_nonzero)
    trace = os.environ.get("BASS_KERNEL_TRACE") == "1"
    tmpdir = os.environ.get("BASS_TRACE_DIR") or None
    res = run_bass_kernel_spmd(nc, in_maps, core_ids=list(range(8)), trace=trace,
                               tmpdir=tmpdir)
    last_exec_time_ns = res.exec_time_ns
    last_bass_results = res

    out = np.empty((B, S, V), np.float32)
    for c in range(8):
        b, q = c // 4, c % 4
        out[b, q * TPC:(q + 1) * TPC, :] = res.results[c]["out"]
    return out
